# revision 13
# baseline (speedup 1.0000x reference)
"""Differentiable K-means (VQ codebook) forward on 8 TRN2 NeuronCores.

x: [16, 8192, 64] f32, centroids: [512, 64] f32
out[n] = softmax_k(-(|x_n - c_k|^2)/T) @ C, T = 0.1

Math: softmax_k(-(x^2 - 2 x.c + c^2)/T) == softmax_k((2 x.c - c^2)/T)
(the x^2 term cancels). E[k,n] = exp((2/T) x_n.c_k); the per-cluster factor
W_k = exp(-c_k^2/T) is folded into mm2's stationary operand.

Device dataflow (per core, 16384 points, 32 tiles of 512):
- mm1 (fp16): cross^T chunks [128 clusters, 512 pts] in PSUM, row-packed
  two-at-a-time in PE row groups 0/64. fp16 keeps every matmul off the slow
  fp32 path and makes weight loads cheap (the v1 kernel spent 66us/core on
  LDWEIGHTS).
- exp split across two engines: ScalarE exact exp (bf16 out) on the first
  1280 columns; VectorE computes the last 768 with a one-op Schraudolph
  fast-exp: i16 = round(s*xc + b) IS the bf16 bit pattern of exp((2/T)xc)
  (the softmax ratio cancels most of the approx error; measured end-to-end
  rel err ~5e-3 vs 2e-2 budget).
- mm2: stationary = cwe chunk [128 clusters, 65] = [W_k*C_k | W_k], moving =
  E chunk [128, 512] -> fin [65, 512] PSUM accumulated over 4 chunks
  (rows 0:64 = unnormalized mixture^T, row 64 = softmax denominator).
- VectorE evacuates fin -> bf16 SBUF, DMA straight to DRAM. The host-side
  unshard divides by the denominator row and transposes back (cheap numpy);
  this keeps the device loop free of any transpose/normalize round-trip.

Sharding: data-parallel on the flattened point axis (131072 -> 8 x 16384),
centroids replicated. No cross-core comms.
"""

from contextlib import ExitStack

import ml_dtypes
import numpy as np

import concourse.bass as bass
import concourse.tile as tile
from concourse import bacc, mybir
from concourse._compat import with_exitstack
from concourse.bass_utils import run_bass_kernel_spmd

N_CORES = 8
N_PTS = 16384  # points per core
K = 512  # clusters
D = 64  # feature dim
TEMP = 0.1
TILE_PTS = 512  # points per inner tile
KC = K // 128  # 4 cluster chunks
CWW = D + 1  # mm2 stationary width: 64 mixture cols + denominator
DVE_COLS = 768  # trailing columns of cr pair1 computed by the DVE fast-exp

F32 = mybir.dt.float32
BF16 = mybir.dt.bfloat16
FP16 = mybir.dt.float16
I16 = mybir.dt.int16

LOG2E = 1.4426950408889634
SCH_S = 128.0 * (2.0 / TEMP) * LOG2E  # bf16-bits per unit of x.c
# 127*128 (exp bias) - sigma correction + 0.5 so truncation rounds to nearest
SCH_SIGMA = 0.0434
SCH_B = 16256.0 - 128.0 * SCH_SIGMA + 0.5


@with_exitstack
def _kmeans_body(ctx: ExitStack, tc: tile.TileContext, out_ap, xt_ap, ct2_ap, cw_ap, n_pts):
    nc = tc.nc
    n_tiles = n_pts // TILE_PTS

    out_r = out_ap.rearrange("(t r) n -> t r n", r=CWW)

    consts = ctx.enter_context(tc.tile_pool(name="consts", bufs=1))
    xtp = ctx.enter_context(tc.tile_pool(name="xtp", bufs=4))
    epool = ctx.enter_context(tc.tile_pool(name="epool", bufs=2))
    fsb = ctx.enter_context(tc.tile_pool(name="fsb", bufs=3))
    ps_cr = ctx.enter_context(tc.tile_pool(name="ps_cr", bufs=3, space="PSUM"))
    ps_fin = ctx.enter_context(tc.tile_pool(name="ps_fin", bufs=2, space="PSUM"))

    # PE warm-up burst + ACT exp-table preload during the startup DMA window.
    wu_in = consts.tile([64, TILE_PTS], FP16)
    nc.vector.memset(wu_in, 1.0)
    wu_ps = ps_fin.tile([CWW, TILE_PTS], F32, tag="fin", name="wu")
    for i in range(3):
        nc.tensor.matmul(
            wu_ps,
            lhsT=wu_in[:, 0:CWW],
            rhs=wu_in,
            start=(i == 0),
            stop=(i == 2),
        )
    wu_e = fsb.tile([CWW, TILE_PTS], BF16, tag="fsb", name="wu_e")
    nc.scalar.activation(wu_e, wu_ps, mybir.ActivationFunctionType.Exp, scale=0.0)

    # constants (DMAs emitted after the first xT load below)
    ct2_sb = consts.tile([128, K], FP16)
    cwe = consts.tile([128, KC * CWW], BF16)

    def load_xt(t):
        # xt DRAM is host-duplicated to [128, n] (rows 64:128 = rows 0:64),
        # one full-width DMA feeds both PE row groups. The first loads ride
        # the Sync HWDGE queue (starts immediately); steady-state loads ride
        # the gpsimd SWDGE queue (whose rings take ~8us to init), issued
        # 3 tiles ahead so DMA+sem latency never stalls mm1.
        xt2 = xtp.tile([128, TILE_PTS], FP16, tag="xt2", name=f"xt2_{t}")
        eng = nc.sync if t < 3 else nc.gpsimd
        eng.dma_start(xt2, xt_ap[:, t * TILE_PTS : (t + 1) * TILE_PTS])
        return xt2

    def mm1_exp(t, xt2):
        toff = 0
        e_sb = epool.tile([128, KC * TILE_PTS], BF16, tag="e", name=f"e_{t}")
        crs = []
        for pair in range(KC // 2):
            cr = ps_cr.tile([128, 2 * TILE_PTS], F32, tag="cr", name=f"cr_{t}_{pair}")
            for h in range(2):
                c = pair * 2 + h
                nc.tensor.matmul(
                    cr[:, h * TILE_PTS : (h + 1) * TILE_PTS],
                    lhsT=ct2_sb[h * 64 : (h + 1) * 64, c * 128 : (c + 1) * 128],
                    rhs=xt2[h * 64 : (h + 1) * 64, toff : toff + TILE_PTS],
                    start=True,
                    stop=True,
                )
            crs.append(cr)
        se_cols = 2 * TILE_PTS - DVE_COLS
        nc.scalar.activation(
            e_sb[:, 0 : 2 * TILE_PTS],
            crs[0],
            mybir.ActivationFunctionType.Exp,
            scale=2.0 / TEMP,
        )
        if se_cols:
            nc.scalar.activation(
                e_sb[:, 2 * TILE_PTS : 2 * TILE_PTS + se_cols],
                crs[1][:, 0:se_cols],
                mybir.ActivationFunctionType.Exp,
                scale=2.0 / TEMP,
            )
        # VectorE fast-exp: build exp's bf16 bit pattern with one fused
        # multiply-add + f32->i16 convert.
        nc.vector.tensor_scalar(
            e_sb[:, 2 * TILE_PTS + se_cols : 4 * TILE_PTS].bitcast(I16),
            crs[1][:, se_cols : 2 * TILE_PTS],
            SCH_S,
            SCH_B,
            mybir.AluOpType.mult,
            mybir.AluOpType.add,
        )
        return e_sb

    def mm2(t, e_sb):
        fin = ps_fin.tile([CWW, TILE_PTS], F32, tag="fin", name=f"fin_{t}")
        for c in range(KC):
            nc.tensor.matmul(
                fin,
                lhsT=cwe[:, c * CWW : (c + 1) * CWW],
                rhs=e_sb[:, c * TILE_PTS : (c + 1) * TILE_PTS],
                start=(c == 0),
                stop=(c == KC - 1),
            )
        return fin

    def evac_store(t, fin):
        # PSUM -> SBUF bf16 evac, then straight to DRAM (unnormalized).
        fsb_t = fsb.tile([CWW, TILE_PTS], BF16, tag="fsb", name=f"fsb_{t}")
        nc.vector.tensor_copy(fsb_t, fin)
        nc.sync.dma_start(out_r[t], fsb_t)

    assert n_tiles >= 4

    # main loop, software-pipelined two tiles deep with 3-deep load prefetch
    nc.sync.dma_start(ct2_sb, ct2_ap)
    nc.sync.dma_start(cwe, cw_ap)
    xt_bufs = {t: load_xt(t) for t in range(min(3, n_tiles))}
    e_bufs = {0: mm1_exp(0, xt_bufs.pop(0))}
    fin_bufs = {}
    for t in range(1, n_tiles + 3):
        if t + 2 < n_tiles:
            xt_bufs[t + 2] = load_xt(t + 2)
        if t < n_tiles:
            e_bufs[t] = mm1_exp(t, xt_bufs.pop(t))
        if t - 1 < n_tiles:
            fin_bufs[t - 1] = mm2(t - 1, e_bufs.pop(t - 1))
        if 0 <= t - 2 < n_tiles:
            evac_store(t - 2, fin_bufs.pop(t - 2))


def build_nc(n_pts=N_PTS, debug=False):
    n_tiles = n_pts // TILE_PTS
    nc = bacc.Bacc("TRN2", target_bir_lowering=False, debug=debug, num_devices=N_CORES)
    xt_in = nc.dram_tensor("xt", [2 * D, n_pts], FP16, kind="ExternalInput").ap()
    ct2_in = nc.dram_tensor("ct2", [128, K], FP16, kind="ExternalInput").ap()
    cw_in = nc.dram_tensor("cw", [128, KC * CWW], BF16, kind="ExternalInput").ap()
    out = nc.dram_tensor("out", [n_tiles * CWW, TILE_PTS], BF16, kind="ExternalOutput").ap()
    with tile.TileContext(nc) as tc:
        _kmeans_body(tc, out, xt_in, ct2_in, cw_in, n_pts)
    nc.compile()
    return nc


def _host_xt(x_shard: np.ndarray) -> np.ndarray:
    """[n, 64] -> fp16 transpose, vertically doubled to [128, n]."""
    xt = x_shard.T.astype(np.float16)
    return np.ascontiguousarray(np.concatenate([xt, xt], axis=0))


def _host_consts(centroids: np.ndarray):
    c = centroids.astype(np.float64)
    ct2 = np.concatenate([centroids.T, centroids.T], axis=0).astype(np.float16)
    w = np.exp(-(c * c).sum(-1) / TEMP)  # [K]
    aug = np.concatenate([c * w[:, None], w[:, None]], axis=1)  # [K, 65]
    cwe = (
        aug.reshape(KC, 128, CWW)
        .transpose(1, 0, 2)
        .reshape(128, KC * CWW)
        .astype(ml_dtypes.bfloat16)
    )
    return np.ascontiguousarray(ct2), np.ascontiguousarray(cwe)


def _host_post(out_dev: np.ndarray) -> np.ndarray:
    """Device output [n_tiles*65, 512] bf16 -> normalized [n, 64] f32."""
    fin = np.asarray(out_dev).astype(np.float32).reshape(-1, CWW, TILE_PTS)
    num = fin[:, :D, :]  # [t, 64, 512]
    den = fin[:, D, :]  # [t, 512]
    o = num / den[:, None, :]
    return o.transpose(0, 2, 1).reshape(-1, D)


_NC_CACHE = None


def kernel(x: np.ndarray, centroids: np.ndarray) -> np.ndarray:
    global _NC_CACHE
    x = np.asarray(x)
    centroids = np.asarray(centroids)
    orig_shape = x.shape
    xf = x.reshape(-1, D).astype(np.float32, copy=False)
    cf = centroids.astype(np.float32, copy=False)
    n_total = xf.shape[0]
    assert n_total == N_CORES * N_PTS, n_total

    if _NC_CACHE is None:
        _NC_CACHE = build_nc()
    nc = _NC_CACHE

    ct2, cwe = _host_consts(cf)
    in_maps = [
        {"xt": _host_xt(xf[i * N_PTS : (i + 1) * N_PTS]), "ct2": ct2, "cw": cwe}
        for i in range(N_CORES)
    ]
    res = run_bass_kernel_spmd(nc, in_maps, core_ids=list(range(N_CORES)))
    out = np.concatenate(
        [_host_post(res.results[i]["out"]) for i in range(N_CORES)], axis=0
    )
    return out.reshape(orig_shape)


# revision 17
# speedup vs baseline: 1.1780x; 1.1780x over previous
"""Differentiable K-means (VQ codebook) forward on 8 TRN2 NeuronCores.

x: [16, 8192, 64] f32, centroids: [512, 64] f32
out[n] = softmax_k(-(|x_n - c_k|^2)/T) @ C, T = 0.1

Math: softmax_k(-(x^2 - 2 x.c + c^2)/T) == softmax_k((2 x.c - c^2)/T)
(the x^2 term cancels). E[k,n] = exp((2/T) x_n.c_k); the per-cluster factor
W_k = exp(-c_k^2/T) is folded into mm2's stationary operand.

Device dataflow (per core, 16384 points, 32 tiles of 512):
- mm1 (fp16): cross^T chunks [128 clusters, 512 pts] in PSUM, row-packed
  two-at-a-time in PE row groups 0/64. fp16 keeps every matmul off the slow
  fp32 path and makes weight loads cheap (the v1 kernel spent 66us/core on
  LDWEIGHTS).
- exp split across two engines: ScalarE exact exp (bf16 out) on the first
  1280 columns; VectorE computes the last 768 with a one-op Schraudolph
  fast-exp: i16 = round(s*xc + b) IS the bf16 bit pattern of exp((2/T)xc)
  (the softmax ratio cancels most of the approx error; measured end-to-end
  rel err ~5e-3 vs 2e-2 budget).
- mm2: stationary = cwe chunk [128 clusters, 65] = [W_k*C_k | W_k], moving =
  E chunk [128, 512] -> fin [65, 512] PSUM accumulated over 4 chunks
  (rows 0:64 = unnormalized mixture^T, row 64 = softmax denominator).
- VectorE evacuates fin -> bf16 SBUF, DMA straight to DRAM. The host-side
  unshard divides by the denominator row and transposes back (cheap numpy);
  this keeps the device loop free of any transpose/normalize round-trip.

Sharding: data-parallel on the flattened point axis (131072 -> 8 x 16384),
centroids replicated. No cross-core comms.
"""

from contextlib import ExitStack

import ml_dtypes
import numpy as np

import concourse.bass as bass
import concourse.tile as tile
from concourse import bacc, mybir
from concourse._compat import with_exitstack
from concourse.bass_utils import run_bass_kernel_spmd

N_CORES = 8
N_PTS = 16384  # points per core
K = 512  # clusters
D = 64  # feature dim
TEMP = 0.1
TILE_PTS = 512  # points per inner tile
KC = K // 128  # 4 cluster chunks
CWW = D + 1  # mm2 stationary width: 64 mixture cols + denominator
DVE_COLS = 768  # trailing columns of cr pair1 computed by the DVE fast-exp

F32 = mybir.dt.float32
BF16 = mybir.dt.bfloat16
FP16 = mybir.dt.float16
I16 = mybir.dt.int16

LOG2E = 1.4426950408889634
SCH_S = 128.0 * (2.0 / TEMP) * LOG2E  # bf16-bits per unit of x.c
# 127*128 (exp bias) - sigma correction + 0.5 so truncation rounds to nearest
SCH_SIGMA = 0.0434
SCH_B = 16256.0 - 128.0 * SCH_SIGMA + 0.5


@with_exitstack
def _kmeans_body(ctx: ExitStack, tc: tile.TileContext, out_ap, xt_ap, ct2_ap, cw_ap, n_pts):
    nc = tc.nc
    n_tiles = n_pts // TILE_PTS

    out_r = out_ap.rearrange("(t r) n -> t r n", r=CWW)

    consts = ctx.enter_context(tc.tile_pool(name="consts", bufs=1))
    xtp = ctx.enter_context(tc.tile_pool(name="xtp", bufs=4))
    epool = ctx.enter_context(tc.tile_pool(name="epool", bufs=3))
    fsb = ctx.enter_context(tc.tile_pool(name="fsb", bufs=3))
    ps_cr = ctx.enter_context(tc.tile_pool(name="ps_cr", bufs=3, space="PSUM"))
    ps_fin = ctx.enter_context(tc.tile_pool(name="ps_fin", bufs=2, space="PSUM"))

    # PE warm-up burst + ACT exp-table preload during the startup DMA window.
    wu_in = consts.tile([64, TILE_PTS], FP16)
    nc.vector.memset(wu_in, 1.0)
    wu_ps = ps_fin.tile([CWW, TILE_PTS], F32, tag="fin", name="wu")
    for i in range(3):
        nc.tensor.matmul(
            wu_ps,
            lhsT=wu_in[:, 0:CWW],
            rhs=wu_in,
            start=(i == 0),
            stop=(i == 2),
        )
    wu_e = fsb.tile([CWW, TILE_PTS], BF16, tag="fsb", name="wu_e")
    nc.scalar.activation(wu_e, wu_ps, mybir.ActivationFunctionType.Exp, scale=0.0)

    # constants (DMAs emitted after the first xT load below)
    ct2_sb = consts.tile([128, K], FP16)
    cwe = consts.tile([128, KC * CWW], BF16)

    def load_group(g):
        # xt DRAM is host-duplicated to [128, n] (rows 64:128 = rows 0:64),
        # one full-width DMA feeds both PE row groups. Two tiles per DMA
        # (fewer triggers = less SBUF contention); the first groups ride the
        # Sync HWDGE queue (starts immediately), later ones the gpsimd SWDGE
        # queue, issued ~4 tiles ahead so DMA+sem latency never stalls mm1.
        t = 2 * g
        xt2 = xtp.tile([128, 2 * TILE_PTS], FP16, tag="xt2", name=f"xt2_{t}")
        eng = nc.sync if g < 2 else nc.gpsimd
        eng.dma_start(xt2, xt_ap[:, t * TILE_PTS : (t + 2) * TILE_PTS])
        return xt2

    def mm1_exp(t, xt2, toff):
        e_sb = epool.tile([128, KC * TILE_PTS], BF16, tag="e", name=f"e_{t}")
        crs = []
        for pair in range(KC // 2):
            cr = ps_cr.tile([128, 2 * TILE_PTS], F32, tag="cr", name=f"cr_{t}_{pair}")
            for h in range(2):
                c = pair * 2 + h
                nc.tensor.matmul(
                    cr[:, h * TILE_PTS : (h + 1) * TILE_PTS],
                    lhsT=ct2_sb[h * 64 : (h + 1) * 64, c * 128 : (c + 1) * 128],
                    rhs=xt2[h * 64 : (h + 1) * 64, toff : toff + TILE_PTS],
                    start=True,
                    stop=True,
                )
            crs.append(cr)
        se_cols = 2 * TILE_PTS - DVE_COLS
        nc.scalar.activation(
            e_sb[:, 0 : 2 * TILE_PTS],
            crs[0],
            mybir.ActivationFunctionType.Exp,
            scale=2.0 / TEMP,
        )
        if se_cols:
            nc.scalar.activation(
                e_sb[:, 2 * TILE_PTS : 2 * TILE_PTS + se_cols],
                crs[1][:, 0:se_cols],
                mybir.ActivationFunctionType.Exp,
                scale=2.0 / TEMP,
            )
        # VectorE fast-exp: build exp's bf16 bit pattern with one fused
        # multiply-add + f32->i16 convert.
        nc.vector.tensor_scalar(
            e_sb[:, 2 * TILE_PTS + se_cols : 4 * TILE_PTS].bitcast(I16),
            crs[1][:, se_cols : 2 * TILE_PTS],
            SCH_S,
            SCH_B,
            mybir.AluOpType.mult,
            mybir.AluOpType.add,
        )
        return e_sb

    def mm2(t, e_sb):
        fin = ps_fin.tile([CWW, TILE_PTS], F32, tag="fin", name=f"fin_{t}")
        for c in range(KC):
            nc.tensor.matmul(
                fin,
                lhsT=cwe[:, c * CWW : (c + 1) * CWW],
                rhs=e_sb[:, c * TILE_PTS : (c + 1) * TILE_PTS],
                start=(c == 0),
                stop=(c == KC - 1),
            )
        return fin

    def evac_store(t, fin):
        # PSUM -> SBUF bf16 evac, then straight to DRAM (unnormalized).
        fsb_t = fsb.tile([CWW, TILE_PTS], BF16, tag="fsb", name=f"fsb_{t}")
        nc.vector.tensor_copy(fsb_t, fin)
        nc.sync.dma_start(out_r[t], fsb_t)

    assert n_tiles >= 4 and n_tiles % 2 == 0
    n_groups = n_tiles // 2

    # main loop, software-pipelined two tiles deep with ~4-tile load prefetch
    nc.sync.dma_start(ct2_sb, ct2_ap)
    nc.sync.dma_start(cwe, cw_ap)
    g_bufs = {g: load_group(g) for g in range(min(3, n_groups))}

    def get_xt(t):
        g = t // 2
        if t % 2 == 0 and g + 3 < n_groups:
            g_bufs[g + 3] = load_group(g + 3)
        return g_bufs[g] if t % 2 == 0 else g_bufs.pop(g), (t % 2) * TILE_PTS

    xt2, toff = get_xt(0)
    e_bufs = {0: mm1_exp(0, xt2, toff)}
    fin_bufs = {}
    for t in range(1, n_tiles + 3):
        if t < n_tiles:
            xt2, toff = get_xt(t)
            e_bufs[t] = mm1_exp(t, xt2, toff)
        if t - 1 < n_tiles:
            fin_bufs[t - 1] = mm2(t - 1, e_bufs.pop(t - 1))
        if 0 <= t - 2 < n_tiles:
            evac_store(t - 2, fin_bufs.pop(t - 2))


def build_nc(n_pts=N_PTS, debug=False):
    n_tiles = n_pts // TILE_PTS
    nc = bacc.Bacc("TRN2", target_bir_lowering=False, debug=debug, num_devices=N_CORES)
    xt_in = nc.dram_tensor("xt", [2 * D, n_pts], FP16, kind="ExternalInput").ap()
    ct2_in = nc.dram_tensor("ct2", [128, K], FP16, kind="ExternalInput").ap()
    cw_in = nc.dram_tensor("cw", [128, KC * CWW], BF16, kind="ExternalInput").ap()
    out = nc.dram_tensor("out", [n_tiles * CWW, TILE_PTS], BF16, kind="ExternalOutput").ap()
    with tile.TileContext(nc) as tc:
        _kmeans_body(tc, out, xt_in, ct2_in, cw_in, n_pts)
    nc.compile()
    return nc


def _host_xt(x_shard: np.ndarray) -> np.ndarray:
    """[n, 64] -> fp16 transpose, vertically doubled to [128, n]."""
    xt = x_shard.T.astype(np.float16)
    return np.ascontiguousarray(np.concatenate([xt, xt], axis=0))


def _host_consts(centroids: np.ndarray):
    c = centroids.astype(np.float64)
    ct2 = np.concatenate([centroids.T, centroids.T], axis=0).astype(np.float16)
    w = np.exp(-(c * c).sum(-1) / TEMP)  # [K]
    aug = np.concatenate([c * w[:, None], w[:, None]], axis=1)  # [K, 65]
    cwe = (
        aug.reshape(KC, 128, CWW)
        .transpose(1, 0, 2)
        .reshape(128, KC * CWW)
        .astype(ml_dtypes.bfloat16)
    )
    return np.ascontiguousarray(ct2), np.ascontiguousarray(cwe)


def _host_post(out_dev: np.ndarray) -> np.ndarray:
    """Device output [n_tiles*65, 512] bf16 -> normalized [n, 64] f32."""
    fin = np.asarray(out_dev).astype(np.float32).reshape(-1, CWW, TILE_PTS)
    num = fin[:, :D, :]  # [t, 64, 512]
    den = fin[:, D, :]  # [t, 512]
    o = num / den[:, None, :]
    return o.transpose(0, 2, 1).reshape(-1, D)


_NC_CACHE = None


def kernel(x: np.ndarray, centroids: np.ndarray) -> np.ndarray:
    global _NC_CACHE
    x = np.asarray(x)
    centroids = np.asarray(centroids)
    orig_shape = x.shape
    xf = x.reshape(-1, D).astype(np.float32, copy=False)
    cf = centroids.astype(np.float32, copy=False)
    n_total = xf.shape[0]
    assert n_total == N_CORES * N_PTS, n_total

    if _NC_CACHE is None:
        _NC_CACHE = build_nc()
    nc = _NC_CACHE

    ct2, cwe = _host_consts(cf)
    in_maps = [
        {"xt": _host_xt(xf[i * N_PTS : (i + 1) * N_PTS]), "ct2": ct2, "cw": cwe}
        for i in range(N_CORES)
    ]
    res = run_bass_kernel_spmd(nc, in_maps, core_ids=list(range(N_CORES)))
    out = np.concatenate(
        [_host_post(res.results[i]["out"]) for i in range(N_CORES)], axis=0
    )
    return out.reshape(orig_shape)


# revision 19
# speedup vs baseline: 1.1970x; 1.0162x over previous
"""Differentiable K-means (VQ codebook) forward on 8 TRN2 NeuronCores.

x: [16, 8192, 64] f32, centroids: [512, 64] f32
out[n] = softmax_k(-(|x_n - c_k|^2)/T) @ C, T = 0.1

Math: softmax_k(-(x^2 - 2 x.c + c^2)/T) == softmax_k((2 x.c - c^2)/T)
(the x^2 term cancels). E[k,n] = exp((2/T) x_n.c_k); the per-cluster factor
W_k = exp(-c_k^2/T) is folded into mm2's stationary operand.

Device dataflow (per core, 16384 points, 32 tiles of 512):
- mm1 (fp16): cross^T chunks [128 clusters, 512 pts] in PSUM, row-packed
  two-at-a-time in PE row groups 0/64. fp16 keeps every matmul off the slow
  fp32 path and makes weight loads cheap (the v1 kernel spent 66us/core on
  LDWEIGHTS).
- exp split across two engines: ScalarE exact exp (bf16 out) on the first
  1280 columns; VectorE computes the last 768 with a one-op Schraudolph
  fast-exp: i16 = round(s*xc + b) IS the bf16 bit pattern of exp((2/T)xc)
  (the softmax ratio cancels most of the approx error; measured end-to-end
  rel err ~5e-3 vs 2e-2 budget).
- mm2: stationary = cwe chunk [128 clusters, 65] = [W_k*C_k | W_k], moving =
  E chunk [128, 512] -> fin [65, 512] PSUM accumulated over 4 chunks
  (rows 0:64 = unnormalized mixture^T, row 64 = softmax denominator).
- VectorE evacuates fin -> bf16 SBUF, DMA straight to DRAM. The host-side
  unshard divides by the denominator row and transposes back (cheap numpy);
  this keeps the device loop free of any transpose/normalize round-trip.

Sharding: data-parallel on the flattened point axis (131072 -> 8 x 16384),
centroids replicated. No cross-core comms.
"""

from contextlib import ExitStack

import ml_dtypes
import numpy as np

import concourse.bass as bass
import concourse.tile as tile
from concourse import bacc, mybir
from concourse._compat import with_exitstack
from concourse.bass_utils import run_bass_kernel_spmd

N_CORES = 8
N_PTS = 16384  # points per core
K = 512  # clusters
D = 64  # feature dim
TEMP = 0.1
TILE_PTS = 512  # points per inner tile
KC = K // 128  # 4 cluster chunks
CWW = D + 1  # mm2 stationary width: 64 mixture cols + denominator
DVE_COLS = 768  # trailing columns of cr pair1 computed by the DVE fast-exp

F32 = mybir.dt.float32
BF16 = mybir.dt.bfloat16
FP16 = mybir.dt.float16
I16 = mybir.dt.int16

LOG2E = 1.4426950408889634
SCH_S = 128.0 * (2.0 / TEMP) * LOG2E  # bf16-bits per unit of x.c
# 127*128 (exp bias) - sigma correction + 0.5 so truncation rounds to nearest
SCH_SIGMA = 0.0434
SCH_B = 16256.0 - 128.0 * SCH_SIGMA + 0.5


@with_exitstack
def _kmeans_body(ctx: ExitStack, tc: tile.TileContext, out_ap, xt_ap, ct2_ap, cw_ap, n_pts):
    nc = tc.nc
    n_tiles = n_pts // TILE_PTS

    out_r = out_ap.rearrange("(t r) n -> t r n", r=CWW)

    consts = ctx.enter_context(tc.tile_pool(name="consts", bufs=1))
    xtp = ctx.enter_context(tc.tile_pool(name="xtp", bufs=4))
    epool = ctx.enter_context(tc.tile_pool(name="epool", bufs=3))
    fsb = ctx.enter_context(tc.tile_pool(name="fsb", bufs=3))
    ps_cr = ctx.enter_context(tc.tile_pool(name="ps_cr", bufs=3, space="PSUM"))
    ps_fin = ctx.enter_context(tc.tile_pool(name="ps_fin", bufs=2, space="PSUM"))

    # PE warm-up burst + ACT exp-table preload during the startup DMA window.
    wu_in = consts.tile([64, TILE_PTS], FP16)
    nc.vector.memset(wu_in, 1.0)
    wu_ps = ps_fin.tile([CWW, TILE_PTS], F32, tag="fin", name="wu")
    for i in range(3):
        nc.tensor.matmul(
            wu_ps,
            lhsT=wu_in[:, 0:CWW],
            rhs=wu_in,
            start=(i == 0),
            stop=(i == 2),
        )
    wu_e = fsb.tile([CWW, TILE_PTS], BF16, tag="fsb", name="wu_e")
    nc.scalar.activation(wu_e, wu_ps, mybir.ActivationFunctionType.Exp, scale=0.0)

    # constants (DMAs emitted after the first xT load below)
    ct2_sb = consts.tile([128, K], FP16)
    cwe = consts.tile([128, KC * CWW], BF16)

    def load_group(g):
        # xt DRAM is host-duplicated to [128, n] (rows 64:128 = rows 0:64),
        # one full-width DMA feeds both PE row groups. Two tiles per DMA
        # (fewer triggers = less SBUF contention); the first groups ride the
        # Sync HWDGE queue (starts immediately), later ones the gpsimd SWDGE
        # queue, issued ~4 tiles ahead so DMA+sem latency never stalls mm1.
        t = 2 * g
        xt2 = xtp.tile([128, 2 * TILE_PTS], FP16, tag="xt2", name=f"xt2_{t}")
        eng = nc.sync if g < 2 else nc.gpsimd
        eng.dma_start(xt2, xt_ap[:, t * TILE_PTS : (t + 2) * TILE_PTS])
        return xt2

    def mm1_exp(t, xt2, toff):
        e_sb = epool.tile([128, KC * TILE_PTS], BF16, tag="e", name=f"e_{t}")
        crs = []
        for pair in range(KC // 2):
            cr = ps_cr.tile([128, 2 * TILE_PTS], F32, tag="cr", name=f"cr_{t}_{pair}")
            for h in range(2):
                c = pair * 2 + h
                nc.tensor.matmul(
                    cr[:, h * TILE_PTS : (h + 1) * TILE_PTS],
                    lhsT=ct2_sb[h * 64 : (h + 1) * 64, c * 128 : (c + 1) * 128],
                    rhs=xt2[h * 64 : (h + 1) * 64, toff : toff + TILE_PTS],
                    start=True,
                    stop=True,
                )
            crs.append(cr)
        se_cols = 2 * TILE_PTS - DVE_COLS
        nc.scalar.activation(
            e_sb[:, 0 : 2 * TILE_PTS],
            crs[0],
            mybir.ActivationFunctionType.Exp,
            scale=2.0 / TEMP,
        )
        if se_cols:
            nc.scalar.activation(
                e_sb[:, 2 * TILE_PTS : 2 * TILE_PTS + se_cols],
                crs[1][:, 0:se_cols],
                mybir.ActivationFunctionType.Exp,
                scale=2.0 / TEMP,
            )
        # VectorE fast-exp: build exp's bf16 bit pattern with one fused
        # multiply-add + f32->i16 convert.
        nc.vector.tensor_scalar(
            e_sb[:, 2 * TILE_PTS + se_cols : 4 * TILE_PTS].bitcast(I16),
            crs[1][:, se_cols : 2 * TILE_PTS],
            SCH_S,
            SCH_B,
            mybir.AluOpType.mult,
            mybir.AluOpType.add,
        )
        return e_sb

    def mm2(t, e_sb):
        fin = ps_fin.tile([CWW, TILE_PTS], F32, tag="fin", name=f"fin_{t}")
        for c in range(KC):
            nc.tensor.matmul(
                fin,
                lhsT=cwe[:, c * CWW : (c + 1) * CWW],
                rhs=e_sb[:, c * TILE_PTS : (c + 1) * TILE_PTS],
                start=(c == 0),
                stop=(c == KC - 1),
            )
        return fin

    def evac_store(t, fin):
        # PSUM -> SBUF bf16 evac, then straight to DRAM (unnormalized).
        fsb_t = fsb.tile([CWW, TILE_PTS], BF16, tag="fsb", name=f"fsb_{t}")
        nc.vector.tensor_copy(fsb_t, fin)
        nc.sync.dma_start(out_r[t], fsb_t)

    assert n_tiles >= 4 and n_tiles % 2 == 0
    n_groups = n_tiles // 2

    # main loop, software-pipelined two tiles deep with ~4-tile load prefetch
    g_bufs = {g: load_group(g) for g in range(min(3, n_groups))}
    nc.gpsimd.dma_start(ct2_sb, ct2_ap)
    nc.gpsimd.dma_start(cwe, cw_ap)

    def get_xt(t):
        g = t // 2
        if t % 2 == 0 and g + 3 < n_groups:
            g_bufs[g + 3] = load_group(g + 3)
        return g_bufs[g] if t % 2 == 0 else g_bufs.pop(g), (t % 2) * TILE_PTS

    xt2, toff = get_xt(0)
    e_bufs = {0: mm1_exp(0, xt2, toff)}
    fin_bufs = {}
    for t in range(1, n_tiles + 3):
        # evac first: its DVE op (CAST) must slot BEFORE this iteration's
        # ts-exp in the DVE program, else the scheduler groups them in
        # pairs and mm2's last chunk stalls every other tile.
        if 0 <= t - 2 < n_tiles:
            evac_store(t - 2, fin_bufs.pop(t - 2))
        if t < n_tiles:
            xt2, toff = get_xt(t)
            e_bufs[t] = mm1_exp(t, xt2, toff)
        if t - 1 < n_tiles:
            fin_bufs[t - 1] = mm2(t - 1, e_bufs.pop(t - 1))


def build_nc(n_pts=N_PTS, debug=False):
    n_tiles = n_pts // TILE_PTS
    nc = bacc.Bacc("TRN2", target_bir_lowering=False, debug=debug, num_devices=N_CORES)
    xt_in = nc.dram_tensor("xt", [2 * D, n_pts], FP16, kind="ExternalInput").ap()
    ct2_in = nc.dram_tensor("ct2", [128, K], FP16, kind="ExternalInput").ap()
    cw_in = nc.dram_tensor("cw", [128, KC * CWW], BF16, kind="ExternalInput").ap()
    out = nc.dram_tensor("out", [n_tiles * CWW, TILE_PTS], BF16, kind="ExternalOutput").ap()
    with tile.TileContext(nc) as tc:
        _kmeans_body(tc, out, xt_in, ct2_in, cw_in, n_pts)
    nc.compile()
    return nc


def _host_xt(x_shard: np.ndarray) -> np.ndarray:
    """[n, 64] -> fp16 transpose, vertically doubled to [128, n]."""
    xt = x_shard.T.astype(np.float16)
    return np.ascontiguousarray(np.concatenate([xt, xt], axis=0))


def _host_consts(centroids: np.ndarray):
    c = centroids.astype(np.float64)
    ct2 = np.concatenate([centroids.T, centroids.T], axis=0).astype(np.float16)
    w = np.exp(-(c * c).sum(-1) / TEMP)  # [K]
    aug = np.concatenate([c * w[:, None], w[:, None]], axis=1)  # [K, 65]
    cwe = (
        aug.reshape(KC, 128, CWW)
        .transpose(1, 0, 2)
        .reshape(128, KC * CWW)
        .astype(ml_dtypes.bfloat16)
    )
    return np.ascontiguousarray(ct2), np.ascontiguousarray(cwe)


def _host_post(out_dev: np.ndarray) -> np.ndarray:
    """Device output [n_tiles*65, 512] bf16 -> normalized [n, 64] f32."""
    fin = np.asarray(out_dev).astype(np.float32).reshape(-1, CWW, TILE_PTS)
    num = fin[:, :D, :]  # [t, 64, 512]
    den = fin[:, D, :]  # [t, 512]
    o = num / den[:, None, :]
    return o.transpose(0, 2, 1).reshape(-1, D)


_NC_CACHE = None


def kernel(x: np.ndarray, centroids: np.ndarray) -> np.ndarray:
    global _NC_CACHE
    x = np.asarray(x)
    centroids = np.asarray(centroids)
    orig_shape = x.shape
    xf = x.reshape(-1, D).astype(np.float32, copy=False)
    cf = centroids.astype(np.float32, copy=False)
    n_total = xf.shape[0]
    assert n_total == N_CORES * N_PTS, n_total

    if _NC_CACHE is None:
        _NC_CACHE = build_nc()
    nc = _NC_CACHE

    ct2, cwe = _host_consts(cf)
    in_maps = [
        {"xt": _host_xt(xf[i * N_PTS : (i + 1) * N_PTS]), "ct2": ct2, "cw": cwe}
        for i in range(N_CORES)
    ]
    res = run_bass_kernel_spmd(nc, in_maps, core_ids=list(range(N_CORES)))
    out = np.concatenate(
        [_host_post(res.results[i]["out"]) for i in range(N_CORES)], axis=0
    )
    return out.reshape(orig_shape)


# revision 21
# speedup vs baseline: 1.3772x; 1.1505x over previous
"""Differentiable K-means (VQ codebook) forward on 8 TRN2 NeuronCores.

x: [16, 8192, 64] f32, centroids: [512, 64] f32
out[n] = softmax_k(-(|x_n - c_k|^2)/T) @ C, T = 0.1

Math: softmax_k(-(x^2 - 2 x.c + c^2)/T) == softmax_k((2 x.c - c^2)/T)
(the x^2 term cancels). E[k,n] = exp((2/T) x_n.c_k); the per-cluster factor
W_k = exp(-c_k^2/T) is folded into mm2's stationary operand.

Device dataflow (per core, 16384 points, 32 tiles of 512):
- mm1 (fp16): cross^T chunks [128 clusters, 512 pts] in PSUM, row-packed
  two-at-a-time in PE row groups 0/64. fp16 keeps every matmul off the slow
  fp32 path and makes weight loads cheap (the v1 kernel spent 66us/core on
  LDWEIGHTS).
- exp split across two engines: ScalarE exact exp (bf16 out) on the first
  1280 columns; VectorE computes the last 768 with a one-op Schraudolph
  fast-exp: i16 = round(s*xc + b) IS the bf16 bit pattern of exp((2/T)xc)
  (the softmax ratio cancels most of the approx error; measured end-to-end
  rel err ~5e-3 vs 2e-2 budget).
- mm2: stationary = cwe chunk [128 clusters, 65] = [W_k*C_k | W_k], moving =
  E chunk [128, 512] -> fin [65, 512] PSUM accumulated over 4 chunks
  (rows 0:64 = unnormalized mixture^T, row 64 = softmax denominator).
- VectorE evacuates fin -> bf16 SBUF, DMA straight to DRAM. The host-side
  unshard divides by the denominator row and transposes back (cheap numpy);
  this keeps the device loop free of any transpose/normalize round-trip.

Sharding: data-parallel on the flattened point axis (131072 -> 8 x 16384),
centroids replicated. No cross-core comms.
"""

from contextlib import ExitStack

import ml_dtypes
import numpy as np

import concourse.bass as bass
import concourse.tile as tile
from concourse import bacc, mybir
from concourse._compat import with_exitstack
from concourse.bass_utils import run_bass_kernel_spmd

N_CORES = 8
N_PTS = 16384  # points per core
K = 512  # clusters
D = 64  # feature dim
TEMP = 0.1
TILE_PTS = 512  # points per inner tile
KC = K // 128  # 4 cluster chunks
CWW = D + 1  # mm2 stationary width: 64 mixture cols + denominator
DVE_COLS = 768  # trailing columns of cr pair1 computed by the DVE fast-exp

F32 = mybir.dt.float32
BF16 = mybir.dt.bfloat16
FP16 = mybir.dt.float16
I16 = mybir.dt.int16

LOG2E = 1.4426950408889634
SCH_S = 128.0 * (2.0 / TEMP) * LOG2E  # bf16-bits per unit of x.c
# 127*128 (exp bias) - sigma correction + 0.5 so truncation rounds to nearest
SCH_SIGMA = 0.0434
SCH_B = 16256.0 - 128.0 * SCH_SIGMA + 0.5


@with_exitstack
def _kmeans_body(ctx: ExitStack, tc: tile.TileContext, out_ap, xt_ap, ct2_ap, cw_ap, n_pts):
    nc = tc.nc
    n_tiles = n_pts // TILE_PTS

    out_r = out_ap.rearrange("(t r) n -> t r n", r=CWW)

    consts = ctx.enter_context(tc.tile_pool(name="consts", bufs=1))
    xtp = ctx.enter_context(tc.tile_pool(name="xtp", bufs=4))
    emain = ctx.enter_context(tc.tile_pool(name="emain", bufs=3))
    edve = ctx.enter_context(tc.tile_pool(name="edve", bufs=3))
    fsb = ctx.enter_context(tc.tile_pool(name="fsb", bufs=3))
    # split cr pools: pair0's reader (ACT#1) finishes early -> 1 buf is
    # enough; pair1's readers (ACT#2 + DVE ts) finish late -> 2 bufs.
    ps_cr0 = ctx.enter_context(tc.tile_pool(name="ps_cr0", bufs=1, space="PSUM"))
    ps_cr1 = ctx.enter_context(tc.tile_pool(name="ps_cr1", bufs=2, space="PSUM"))
    ps_fin = ctx.enter_context(tc.tile_pool(name="ps_fin", bufs=2, space="PSUM"))

    # PE warm-up burst + ACT exp-table preload during the startup DMA window.
    wu_in = consts.tile([64, TILE_PTS], FP16)
    nc.vector.memset(wu_in, 1.0)
    wu_ps = ps_fin.tile([CWW, TILE_PTS], F32, tag="fin", name="wu")
    for i in range(3):
        nc.tensor.matmul(
            wu_ps,
            lhsT=wu_in[:, 0:CWW],
            rhs=wu_in,
            start=(i == 0),
            stop=(i == 2),
        )
    wu_e = fsb.tile([CWW, TILE_PTS], BF16, tag="fsb", name="wu_e")
    nc.scalar.activation(wu_e, wu_ps, mybir.ActivationFunctionType.Exp, scale=0.0)

    # constants (DMAs emitted after the first xT load below)
    ct2_sb = consts.tile([128, K], FP16)
    cwe = consts.tile([128, KC * CWW], BF16)

    def load_group(g):
        # xt DRAM is host-duplicated to [128, n] (rows 64:128 = rows 0:64),
        # one full-width DMA feeds both PE row groups. Two tiles per DMA
        # (fewer triggers = less SBUF contention); the first groups ride the
        # Sync HWDGE queue (starts immediately), later ones the gpsimd SWDGE
        # queue, issued ~4 tiles ahead so DMA+sem latency never stalls mm1.
        t = 2 * g
        xt2 = xtp.tile([128, 2 * TILE_PTS], FP16, tag="xt2", name=f"xt2_{t}")
        eng = nc.sync if g < 2 else nc.gpsimd
        eng.dma_start(xt2, xt_ap[:, t * TILE_PTS : (t + 2) * TILE_PTS])
        return xt2

    se_cols = 2 * TILE_PTS - DVE_COLS  # columns of pair1 done by ScalarE

    def mm1_exp(t, xt2, toff):
        # Two e-tiles so the ScalarE and VectorE exp writers share no tile:
        # the DVE fast-exp runs in parallel with the ACTs instead of behind
        # a false write-write dependency.
        e_m = emain.tile([128, 2 * TILE_PTS + se_cols], BF16, tag="em", name=f"em_{t}")
        e_d = edve.tile([128, DVE_COLS], I16, tag="ed", name=f"ed_{t}")
        crs = []
        for pair in range(KC // 2):
            pool = ps_cr0 if pair == 0 else ps_cr1
            cr = pool.tile([128, 2 * TILE_PTS], F32, tag=f"cr{pair}", name=f"cr_{t}_{pair}")
            for h in range(2):
                c = pair * 2 + h
                nc.tensor.matmul(
                    cr[:, h * TILE_PTS : (h + 1) * TILE_PTS],
                    lhsT=ct2_sb[h * 64 : (h + 1) * 64, c * 128 : (c + 1) * 128],
                    rhs=xt2[h * 64 : (h + 1) * 64, toff : toff + TILE_PTS],
                    start=True,
                    stop=True,
                )
            crs.append(cr)
        # VectorE fast-exp first (independent of the ACTs): exp's bf16 bit
        # pattern via one fused multiply-add + f32->i16 convert.
        nc.vector.tensor_scalar(
            e_d,
            crs[1][:, se_cols : 2 * TILE_PTS],
            SCH_S,
            SCH_B,
            mybir.AluOpType.mult,
            mybir.AluOpType.add,
        )
        nc.scalar.activation(
            e_m[:, 0 : 2 * TILE_PTS],
            crs[0],
            mybir.ActivationFunctionType.Exp,
            scale=2.0 / TEMP,
        )
        if se_cols:
            nc.scalar.activation(
                e_m[:, 2 * TILE_PTS : 2 * TILE_PTS + se_cols],
                crs[1][:, 0:se_cols],
                mybir.ActivationFunctionType.Exp,
                scale=2.0 / TEMP,
            )
        return e_m, e_d

    def mm2(t, ebufs):
        e_m, e_d = ebufs
        e_db = e_d[:].bitcast(BF16)
        fin = ps_fin.tile([CWW, TILE_PTS], F32, tag="fin", name=f"fin_{t}")
        # rhs per chunk c = exp columns [c*512, (c+1)*512); chunk 2 straddles
        # the ScalarE/VectorE boundary so it runs as two column-halves.
        nc.tensor.matmul(fin, lhsT=cwe[:, 0:CWW], rhs=e_m[:, 0:TILE_PTS],
                         start=True, stop=False)
        nc.tensor.matmul(fin, lhsT=cwe[:, CWW : 2 * CWW],
                         rhs=e_m[:, TILE_PTS : 2 * TILE_PTS],
                         start=False, stop=False)
        nc.tensor.matmul(fin[:, 0:se_cols], lhsT=cwe[:, 2 * CWW : 3 * CWW],
                         rhs=e_m[:, 2 * TILE_PTS : 2 * TILE_PTS + se_cols],
                         start=False, stop=False)
        nc.tensor.matmul(fin[:, se_cols:TILE_PTS], lhsT=cwe[:, 2 * CWW : 3 * CWW],
                         rhs=e_db[:, 0 : TILE_PTS - se_cols],
                         start=False, stop=False)
        nc.tensor.matmul(fin, lhsT=cwe[:, 3 * CWW : 4 * CWW],
                         rhs=e_db[:, TILE_PTS - se_cols : DVE_COLS],
                         start=False, stop=True)
        return fin

    def evac_store(t, fin):
        # PSUM -> SBUF bf16 evac, then straight to DRAM (unnormalized).
        fsb_t = fsb.tile([CWW, TILE_PTS], BF16, tag="fsb", name=f"fsb_{t}")
        nc.vector.tensor_copy(fsb_t, fin)
        nc.sync.dma_start(out_r[t], fsb_t)

    assert n_tiles >= 4 and n_tiles % 2 == 0
    n_groups = n_tiles // 2

    # main loop, software-pipelined two tiles deep with ~4-tile load prefetch
    g_bufs = {g: load_group(g) for g in range(min(3, n_groups))}
    nc.gpsimd.dma_start(ct2_sb, ct2_ap)
    nc.gpsimd.dma_start(cwe, cw_ap)

    def get_xt(t):
        g = t // 2
        if t % 2 == 0 and g + 3 < n_groups:
            g_bufs[g + 3] = load_group(g + 3)
        return g_bufs[g] if t % 2 == 0 else g_bufs.pop(g), (t % 2) * TILE_PTS

    xt2, toff = get_xt(0)
    e_bufs = {0: mm1_exp(0, xt2, toff)}
    fin_bufs = {}
    for t in range(1, n_tiles + 3):
        # evac first: its DVE op (CAST) must slot BEFORE this iteration's
        # ts-exp in the DVE program, else the scheduler groups them in
        # pairs and mm2's last chunk stalls every other tile.
        if 0 <= t - 2 < n_tiles:
            evac_store(t - 2, fin_bufs.pop(t - 2))
        if t < n_tiles:
            xt2, toff = get_xt(t)
            e_bufs[t] = mm1_exp(t, xt2, toff)
        if t - 1 < n_tiles:
            fin_bufs[t - 1] = mm2(t - 1, e_bufs.pop(t - 1))


def build_nc(n_pts=N_PTS, debug=False):
    n_tiles = n_pts // TILE_PTS
    nc = bacc.Bacc("TRN2", target_bir_lowering=False, debug=debug, num_devices=N_CORES)
    xt_in = nc.dram_tensor("xt", [2 * D, n_pts], FP16, kind="ExternalInput").ap()
    ct2_in = nc.dram_tensor("ct2", [128, K], FP16, kind="ExternalInput").ap()
    cw_in = nc.dram_tensor("cw", [128, KC * CWW], BF16, kind="ExternalInput").ap()
    out = nc.dram_tensor("out", [n_tiles * CWW, TILE_PTS], BF16, kind="ExternalOutput").ap()
    with tile.TileContext(nc) as tc:
        _kmeans_body(tc, out, xt_in, ct2_in, cw_in, n_pts)
    nc.compile()
    return nc


def _host_xt(x_shard: np.ndarray) -> np.ndarray:
    """[n, 64] -> fp16 transpose, vertically doubled to [128, n]."""
    xt = x_shard.T.astype(np.float16)
    return np.ascontiguousarray(np.concatenate([xt, xt], axis=0))


def _host_consts(centroids: np.ndarray):
    c = centroids.astype(np.float64)
    ct2 = np.concatenate([centroids.T, centroids.T], axis=0).astype(np.float16)
    w = np.exp(-(c * c).sum(-1) / TEMP)  # [K]
    aug = np.concatenate([c * w[:, None], w[:, None]], axis=1)  # [K, 65]
    cwe = (
        aug.reshape(KC, 128, CWW)
        .transpose(1, 0, 2)
        .reshape(128, KC * CWW)
        .astype(ml_dtypes.bfloat16)
    )
    return np.ascontiguousarray(ct2), np.ascontiguousarray(cwe)


def _host_post(out_dev: np.ndarray) -> np.ndarray:
    """Device output [n_tiles*65, 512] bf16 -> normalized [n, 64] f32."""
    fin = np.asarray(out_dev).astype(np.float32).reshape(-1, CWW, TILE_PTS)
    num = fin[:, :D, :]  # [t, 64, 512]
    den = fin[:, D, :]  # [t, 512]
    o = num / den[:, None, :]
    return o.transpose(0, 2, 1).reshape(-1, D)


_NC_CACHE = None


def kernel(x: np.ndarray, centroids: np.ndarray) -> np.ndarray:
    global _NC_CACHE
    x = np.asarray(x)
    centroids = np.asarray(centroids)
    orig_shape = x.shape
    xf = x.reshape(-1, D).astype(np.float32, copy=False)
    cf = centroids.astype(np.float32, copy=False)
    n_total = xf.shape[0]
    assert n_total == N_CORES * N_PTS, n_total

    if _NC_CACHE is None:
        _NC_CACHE = build_nc()
    nc = _NC_CACHE

    ct2, cwe = _host_consts(cf)
    in_maps = [
        {"xt": _host_xt(xf[i * N_PTS : (i + 1) * N_PTS]), "ct2": ct2, "cw": cwe}
        for i in range(N_CORES)
    ]
    res = run_bass_kernel_spmd(nc, in_maps, core_ids=list(range(N_CORES)))
    out = np.concatenate(
        [_host_post(res.results[i]["out"]) for i in range(N_CORES)], axis=0
    )
    return out.reshape(orig_shape)
